# revision 40
# baseline (speedup 1.0000x reference)
"""Trainium2 Bass kernel for nn_MultiHeadAttention_9912784519532.

MHA with relative position bias: b=2, n=2048, dim=512, heads=8, d_head=64,
rel table (2*512+1, 64).

Sharding: 16 (batch, head) pairs over 8 cores -> 2 heads of one batch per
core. Each core computes a partial output y_part = attn_out @ Wo_slice for
its 2 heads; host sums 4 partials per batch and adds bo.

Per-core algorithm (keys-on-partitions / transposed-attention orientation):
  qT/kT = W.T @ x.T via PE (2 heads packed on partitions 0-63 / 64-127)
  ktp = kT + rel_emb[1024] (fold the past-edge bias into k for ALL
    non-future blocks); ktf = kT + rel_emb[0] (future blocks)
  wER[n, c] = q[n] . relX'[c] where relX'[c] =
    rel_emb[clip(1152 - c, 0, 1024)] - rel_emb[1024]  (so window blocks
    add only the *residual* bias on top of ktp; left pad is exactly 0,
    right pad is rel_emb[0]-rel_emb[1024]) -> DRAM scratch
  For each (q-chunk 1024, head, key-tile 128):
    Z^T = kvar.T @ qT  (ktf for fully-future 128-blocks, ktp otherwise)
    window blocks (|block delta| <= 4): Z^T += transpose-matmul of
      diagonally-DMA'd wER tiles (a plain 2D-strided DRAM read)
    attnT = exp(0.125 * Z^T)  (ScalarE; logits are O(1), no max needed)
    outT_aug += [v | 1].T @ attnT  (row 64 accumulates the softmax denom)
  Per (h, half) slot: rden = 1/den via fast DVE reciprocal; broadcast via
  a K=1 float32r matmul; otn = outT * rden; y_nt = sum_h otn_h.T @ Wo_h.

All PE operands are bf16 (fp32 runs at 4 cycles/row on the PE, bf16 at 1);
accumulation stays fp32 in PSUM, and the final y is fp32.
"""

import numpy as np

HEADS = 8
D = 64
N = 2048
DIM = 512
WER = 1280  # width of padded/reversed rel projection table
P = 128

_cached = {}


def _build_program():
    import concourse.bass as bass
    import concourse.mybir as mybir
    import concourse.tile as tile
    from concourse import bacc

    f32 = mybir.dt.float32
    f32r = mybir.dt.float32r
    bf16 = mybir.dt.bfloat16
    AP = bass.AP

    nc = bacc.Bacc(
        "TRN2",
        target_bir_lowering=False,
        debug=False,
        enable_asserts=False,
        num_devices=8,
    )

    xT_d = nc.dram_tensor("xT", [DIM, N], bf16, kind="ExternalInput")
    wq_d = nc.dram_tensor("wq2", [DIM, P], bf16, kind="ExternalInput")
    wk_d = nc.dram_tensor("wk2", [DIM, P], bf16, kind="ExternalInput")
    wv_d = nc.dram_tensor("wv2", [DIM, P], bf16, kind="ExternalInput")
    wo_d = nc.dram_tensor("wo2", [P, DIM], bf16, kind="ExternalInput")
    relx_d = nc.dram_tensor("relx2", [P, WER], bf16, kind="ExternalInput")
    edge_d = nc.dram_tensor("edge2", [P, 2], f32, kind="ExternalInput")
    ident_d = nc.dram_tensor("ident", [P, P], f32, kind="ExternalInput")
    y_d = nc.dram_tensor("y", [N, DIM], f32, kind="ExternalOutput")

    wer_d = [
        nc.dram_tensor(f"wer{h}", [N, WER], bf16, kind="Internal") for h in range(2)
    ]

    SCALE = float(D) ** -0.5
    NT = N // P  # 16 key tiles
    QW = 1024  # q-chunk width for the flash loop

    with tile.TileContext(nc) as tc:
        import contextlib

        ctx = contextlib.ExitStack()
        with ctx:
            const = ctx.enter_context(tc.tile_pool(name="const", bufs=1))
            big = ctx.enter_context(tc.tile_pool(name="big", bufs=1))
            cpool = ctx.enter_context(tc.tile_pool(name="copies", bufs=4))
            # PSUM budget (8 banks): zt pool 2x[128,1024]=4 + gpool 4x 1-bank=4.
            # zt has its own pool so flash never blocks on the pre-flash phases.
            ppool = ctx.enter_context(tc.tile_pool(name="ps", bufs=2, space="PSUM"))
            opool = ctx.enter_context(tc.tile_pool(name="ot", bufs=4, space="PSUM"))
            apool = ctx.enter_context(tc.tile_pool(name="attn", bufs=3))
            wpool = ctx.enter_context(tc.tile_pool(name="win", bufs=12))
            spool = ctx.enter_context(tc.tile_pool(name="small", bufs=2))
            dpool = ctx.enter_context(tc.tile_pool(name="dens", bufs=2))
            rbpool = ctx.enter_context(tc.tile_pool(name="rbp", bufs=2))

            # ---- load constants / inputs (small weights first) ----
            wq_sb = const.tile([P, 4, P], bf16)
            nc.sync.dma_start(wq_sb[:], wq_d.ap().rearrange("(c p) m -> p c m", p=P))
            wk_sb = const.tile([P, 4, P], bf16)
            nc.sync.dma_start(wk_sb[:], wk_d.ap().rearrange("(c p) m -> p c m", p=P))
            wv_sb = const.tile([P, 4, P], bf16)
            nc.sync.dma_start(wv_sb[:], wv_d.ap().rearrange("(c p) m -> p c m", p=P))
            wo_sb = const.tile([64, 2, DIM], bf16)
            nc.sync.dma_start(wo_sb[:], wo_d.ap().rearrange("(h p) m -> p h m", p=64))
            relx_sb = const.tile([P, WER], bf16)
            nc.sync.dma_start(relx_sb[:], relx_d.ap())
            edge_sb = const.tile([P, 2], f32)
            nc.sync.dma_start(edge_sb[:], edge_d.ap())
            ident_sb = const.tile([P, P], f32)
            nc.sync.dma_start(ident_sb[:], ident_d.ap())
            zero_sb = const.tile([P, P], bf16)
            nc.vector.memset(zero_sb[:], 0.0)
            den_pack = const.tile([97, 2, 512], f32)
            nc.vector.memset(den_pack[:], 1.0)
            ones64 = const.tile([P, 64], f32)
            nc.vector.memset(ones64[:], 1.0)

            # xT in 4 column chunks so projections can start early
            xt_sb = big.tile([P, 4, N], bf16)
            for ch in range(4):
                nc.sync.dma_start(
                    xt_sb[:, :, ch * 512 : (ch + 1) * 512],
                    xT_d.ap()[:, ch * 512 : (ch + 1) * 512].rearrange(
                        "(c p) n -> p c n", p=P
                    ),
                )

            # wER columns [0,128) are identically zero: write them once (on
            # the idle gpsimd queue, so the xT load is not delayed).
            for h in range(2):
                for qt in range(NT):
                    nc.gpsimd.dma_start(
                        wer_d[h].ap()[qt * P : (qt + 1) * P, 0:128], zero_sb[:]
                    )

            # ---- projections: qT2/kT2 (2 heads packed on partitions) ----
            qt2 = big.tile([P, N], bf16)
            ktp = big.tile([P, N], bf16)
            ktf = big.tile([P, N], bf16)
            kt_raw = big.tile([P, N], bf16)
            # Interleave q/k/v projections per n-chunk so PE work tracks the
            # chunked xT arrival instead of stalling on later chunks.
            v2 = big.tile([P, 2, NT, 65], bf16)
            nc.vector.memset(v2[:], 1.0)
            for nch in range(4):
                for dst, wsb in ((qt2, wq_sb), (kt_raw, wk_sb)):
                    pt = opool.tile([P, 512], f32, name="proj", tag="outT")
                    for cc in range(4):
                        nc.tensor.matmul(
                            pt[:],
                            wsb[:, cc, :],
                            xt_sb[:, cc, nch * 512 : (nch + 1) * 512],
                            start=(cc == 0),
                            stop=(cc == 3),
                        )
                    nc.vector.tensor_copy(
                        dst[:, nch * 512 : (nch + 1) * 512], pt[:]
                    )
                for kt in range(nch * 4, nch * 4 + 4):
                    pt = opool.tile([P, 512], f32, name="vproj", tag="outT")
                    for cc in range(4):
                        nc.tensor.matmul(
                            pt[:, :P],
                            xt_sb[:, cc, kt * P : (kt + 1) * P],
                            wv_sb[:, cc, :],
                            start=(cc == 0),
                            stop=(cc == 3),
                        )
                    for h in range(2):
                        nc.vector.tensor_copy(
                            v2[:, h, kt, 0:64], pt[:, h * 64 : h * 64 + 64]
                        )

            nc.vector.tensor_scalar_add(ktp[:], kt_raw[:], edge_sb[:, 0:1])
            nc.vector.tensor_scalar_add(ktf[:], kt_raw[:], edge_sb[:, 1:2])

            # ---- wER tables -> DRAM.
            # relX' columns [0,128) are exactly zero (pre-zeroed above);
            # [1152,1280) give the future-minus-past edge residual.
            # h0's tables are emitted before flash(0,0); h1's groups are
            # interleaved into flash(0,0)'s kt loop as background work so the
            # PE stream stays dense and flash starts ~15us earlier.
            def wer_group(h, qt):
                hs = slice(h * 64, h * 64 + 64)
                wtile = cpool.tile([P, 1152], bf16, name="wer_sb", tag="wer_sb")
                ptA1 = opool.tile([P, 512], f32, name="wer_psA1", tag="outT")
                nc.tensor.matmul(
                    ptA1[:],
                    qt2[hs, qt * P : (qt + 1) * P],
                    relx_sb[hs, 128:640],
                    start=True,
                    stop=True,
                )
                ptA2 = opool.tile([P, 512], f32, name="wer_psA2", tag="outT")
                nc.tensor.matmul(
                    ptA2[:],
                    qt2[hs, qt * P : (qt + 1) * P],
                    relx_sb[hs, 640:1152],
                    start=True,
                    stop=True,
                )
                nc.vector.tensor_copy(wtile[:, 0:512], ptA1[:])
                if h == 0:
                    nc.scalar.copy(wtile[:, 512:1024], ptA2[:])
                else:
                    nc.vector.tensor_copy(wtile[:, 512:1024], ptA2[:])
                ptB = opool.tile([P, 512], f32, name="wer_psB", tag="outT")
                nc.tensor.matmul(
                    ptB[:, 0:128],
                    qt2[hs, qt * P : (qt + 1) * P],
                    relx_sb[hs, 1152:1280],
                    start=True,
                    stop=True,
                )
                if h == 0:
                    nc.vector.tensor_copy(wtile[:, 1024:1152], ptB[:, 0:128])
                else:
                    nc.scalar.copy(wtile[:, 1024:1152], ptB[:, 0:128])
                nc.sync.dma_start(
                    wer_d[h].ap()[qt * P : (qt + 1) * P, 128:1280], wtile[:]
                )

            for qt in range(NT):
                wer_group(0, qt)

            # ---- flash attention (qc outer so the tail pipelines) ----
            otn = big.tile([64, 2, N], bf16)  # normalized outT per head
            ostages = {}

            def flash(qc, h, bg=()):
                bg = list(bg)
                hs = slice(h * 64, h * 64 + 64)
                if h == 0:
                    ostages[qc] = spool.tile(
                        [65, 4, 512], f32, name="ostage", tag="ostage"
                    )
                ostage = ostages[qc]
                # Window bias tiles: diagonally-gathered rows of the wER
                # table (contiguous 2.3KB per-row reads, cast bf16->f32 by
                # the SWDGE path); consumed by PE transpose-matmuls.
                wins = []
                for j in range(QW // P):
                    qb = qc * QW + j * P
                    r0 = max(0, qb - 512)
                    r1 = min(N, qb + 640)
                    rw = r1 - r0
                    wt = wpool.tile([P, 1152], f32, name=f"win{j}", tag="win")
                    wsrc = AP(
                        tensor=wer_d[h],
                        offset=qb * (WER - 1) + 640 + r0,
                        ap=[[WER - 1, P], [1, rw]],
                    )
                    nc.gpsimd.dma_start(wt[:, :rw], wsrc)
                    wins.append((wt, r0))

                oth = [
                    opool.tile([65, 512], f32, name=f"outT{half}", tag="outT")
                    for half in range(2)
                ]
                for kt in range(NT):
                    kb = kt * P
                    zt = ppool.tile([P, QW], f32, name="zt", tag="ps")
                    # class per 128-block: fully-future -> ktf, else ktp
                    cls = []
                    for j in range(QW // P):
                        dlt = qc * QW + j * P - kb
                        cls.append("f" if dlt <= -640 else "u")
                    for half in range(QW // 512):
                        j0 = half * 4
                        runs = []
                        for j in range(j0, j0 + 4):
                            if runs and runs[-1][2] == cls[j]:
                                runs[-1][1] += P
                            else:
                                runs.append([j * P, P, cls[j]])
                        first = True
                        for s, wd, c in runs:
                            kvar = ktf if c == "f" else ktp
                            nc.tensor.matmul(
                                zt[:, s : s + wd],
                                kvar[hs, kb : kb + P],
                                qt2[hs, qc * QW + s : qc * QW + s + wd],
                                start=first,
                                stop=False,
                                skip_group_check=True,
                            )
                            first = False
                        for j in range(j0, j0 + 4):
                            dlt = qc * QW + j * P - kb
                            if abs(dlt) >= 640:
                                continue
                            wt, r0 = wins[j]
                            nc.tensor.matmul(
                                zt[:, j * P : (j + 1) * P],
                                wt[:, kb - r0 : kb - r0 + P],
                                ident_sb[:],
                                is_transpose=True,
                                start=False,
                                stop=False,
                                skip_group_check=True,
                            )
                    at = apool.tile([P, QW], bf16, name="attnT")
                    nc.scalar.activation(
                        at[:], zt[:], mybir.ActivationFunctionType.Exp,
                        scale=SCALE,
                    )
                    for half in range(2):
                        nc.tensor.matmul(
                            oth[half][:],
                            v2[:, h, kt, :],
                            at[:, half * 512 : (half + 1) * 512],
                            start=(kt == 0),
                            stop=(kt == NT - 1),
                        )
                    # up to 2 units of background work (h1 wER groups) per kt
                    for _ in range(2):
                        if bg:
                            bg.pop(0)()
                for half in range(2):
                    slot = h * 2 + half
                    nc.vector.tensor_copy(ostage[:, slot, :], oth[half][:])

            def tail(qc):
                # Pack the 4 denominator rows at partitions 0/32/64/96 so one
                # batched reciprocal covers them (recip cost is per-partition-
                # element; a [1,512] alone costs the same 3.4us). The
                # broadcast back to 64 partitions rides the idle GpSimd.
                ostage = ostages.pop(qc)
                for slot in range(4):
                    nc.sync.dma_start(
                        den_pack[32 * slot : 32 * slot + 1, qc, :],
                        ostage[64:65, slot, :],
                    )
                rden_pack = dpool.tile([97, 512], f32, name="rdenp", tag="den")
                nc.vector.reciprocal(rden_pack[:], den_pack[:, qc, :])
                for slot in range(4):
                    h, half = slot // 2, slot % 2
                    rb = opool.tile([64, 512], f32, name="recipb", tag="outT")
                    nc.tensor.matmul(
                        rb[:],
                        ones64[32 * slot : 32 * slot + 1, :],
                        rden_pack[32 * slot : 32 * slot + 1, :],
                        start=True,
                        stop=True,
                        tile_position=(32 * slot, 0),
                    )
                    q0 = qc * QW + half * 512
                    nc.vector.tensor_mul(
                        otn[:, h, q0 : q0 + 512], ostage[0:64, slot, :], rb[:]
                    )
                for nt8 in range(QW // P):
                    nt = qc * (QW // P) + nt8
                    pt = opool.tile([P, 512], f32, name="yproj", tag="outT")
                    for h in range(2):
                        nc.tensor.matmul(
                            pt[:],
                            otn[:, h, nt * P : (nt + 1) * P],
                            wo_sb[:, h, :],
                            start=(h == 0),
                            stop=(h == 1),
                        )
                    yt = cpool.tile([P, 512], f32, name="y_sb")
                    nc.vector.tensor_copy(yt[:], pt[:])
                    nc.sync.dma_start(y_d.ap()[nt * P : (nt + 1) * P, :], yt[:])

            # tail(0) is emitted after flash(1,0) so the PE never waits on the
            # reciprocal chain (PE executes strictly in emission order).
            flash(0, 0, bg=[
                (lambda qt=qt: wer_group(1, qt)) for qt in range(NT)
            ])
            flash(0, 1)
            flash(1, 0)
            tail(0)
            flash(1, 1)
            tail(1)

    nc.compile()
    return nc


def _host_prep(x, Wq, Wkv, Wo, rel_emb):
    """Build the 8 per-core input maps."""
    import ml_dtypes

    bf = ml_dtypes.bfloat16
    ident = np.eye(P, dtype=np.float32)
    relX = (
        rel_emb[np.clip(1152 - np.arange(WER), 0, 1024)] - rel_emb[1024][None, :]
    ).T
    relx2 = np.ascontiguousarray(np.concatenate([relX, relX], axis=0).astype(bf))
    edge = np.stack([rel_emb[1024], rel_emb[0]], axis=1)
    edge2 = np.ascontiguousarray(
        np.concatenate([edge, edge], axis=0).astype(np.float32)
    )
    Wkv_r = Wkv.reshape(DIM, 2, HEADS, D)
    in_maps = []
    for core in range(8):
        b = core // 4
        h0 = 2 * (core % 4)
        in_maps.append(
            {
                "xT": np.ascontiguousarray(x[b].T.astype(bf)),
                "wq2": np.ascontiguousarray(Wq[:, h0 * D : (h0 + 2) * D].astype(bf)),
                "wk2": np.ascontiguousarray(
                    Wkv_r[:, 0, h0 : h0 + 2].reshape(DIM, 2 * D).astype(bf)
                ),
                "wv2": np.ascontiguousarray(
                    Wkv_r[:, 1, h0 : h0 + 2].reshape(DIM, 2 * D).astype(bf)
                ),
                "wo2": np.ascontiguousarray(
                    Wo[h0 * D : (h0 + 2) * D, :].astype(bf)
                ),
                "relx2": relx2,
                "edge2": edge2,
                "ident": ident,
            }
        )
    return in_maps


def kernel(x, Wq, Wkv, Wo, bo, rel_emb, _want_trace=False):
    from concourse.bass_utils import run_bass_kernel_spmd

    x = np.asarray(x)
    if "nc" not in _cached:
        _cached["nc"] = _build_program()
    nc = _cached["nc"]
    in_maps = _host_prep(x, np.asarray(Wq), np.asarray(Wkv), np.asarray(Wo),
                         np.asarray(rel_emb))
    res = run_bass_kernel_spmd(
        nc, in_maps, core_ids=list(range(8)), trace=_want_trace
    )
    _cached["last_result"] = res
    y = np.zeros((2, N, DIM), np.float32)
    for core in range(8):
        y[core // 4] += res.results[core]["y"]
    y += np.asarray(bo).astype(np.float32)[None, None, :]
    return y


# revision 43
# speedup vs baseline: 1.0917x; 1.0917x over previous
"""Trainium2 Bass kernel for nn_MultiHeadAttention_9912784519532.

MHA with relative position bias: b=2, n=2048, dim=512, heads=8, d_head=64,
rel table (2*512+1, 64).

Sharding: 16 (batch, head) pairs over 8 cores -> 2 heads of one batch per
core. Each core computes a partial output y_part = attn_out @ Wo_slice for
its 2 heads; host sums 4 partials per batch and adds bo.

Per-core algorithm (keys-on-partitions / transposed-attention orientation):
  qT/kT = W.T @ x.T via PE (2 heads packed on partitions 0-63 / 64-127)
  ktp = kT + rel_emb[1024] (fold the past-edge bias into k for ALL
    non-future blocks); ktf = kT + rel_emb[0] (future blocks)
  wER[n, c] = q[n] . relX'[c] where relX'[c] =
    rel_emb[clip(1152 - c, 0, 1024)] - rel_emb[1024]  (so window blocks
    add only the *residual* bias on top of ktp; left pad is exactly 0,
    right pad is rel_emb[0]-rel_emb[1024]) -> DRAM scratch
  For each (q-chunk 1024, head, key-tile 128):
    Z^T = kvar.T @ qT  (ktf for fully-future 128-blocks, ktp otherwise)
    window blocks (|block delta| <= 4): Z^T += transpose-matmul of
      diagonally-DMA'd wER tiles (a plain 2D-strided DRAM read)
    attnT = exp(0.125 * Z^T)  (ScalarE; logits are O(1), no max needed)
    outT_aug += [v | 1].T @ attnT  (row 64 accumulates the softmax denom)
  Per (h, half) slot: rden = 1/den via fast DVE reciprocal; broadcast via
  a K=1 float32r matmul; otn = outT * rden; y_nt = sum_h otn_h.T @ Wo_h.

All PE operands are bf16 (fp32 runs at 4 cycles/row on the PE, bf16 at 1);
accumulation stays fp32 in PSUM, and the final y is fp32.
"""

import numpy as np

HEADS = 8
D = 64
N = 2048
DIM = 512
WER = 1280  # width of padded/reversed rel projection table
P = 128

_cached = {}


def _build_program():
    import concourse.bass as bass
    import concourse.mybir as mybir
    import concourse.tile as tile
    from concourse import bacc

    f32 = mybir.dt.float32
    f32r = mybir.dt.float32r
    bf16 = mybir.dt.bfloat16
    AP = bass.AP

    nc = bacc.Bacc(
        "TRN2",
        target_bir_lowering=False,
        debug=False,
        enable_asserts=False,
        num_devices=8,
    )

    xT_d = nc.dram_tensor("xT", [DIM, N], bf16, kind="ExternalInput")
    wq_d = nc.dram_tensor("wq2", [DIM, P], bf16, kind="ExternalInput")
    wk_d = nc.dram_tensor("wk2", [DIM, P], bf16, kind="ExternalInput")
    wv_d = nc.dram_tensor("wv2", [DIM, P], bf16, kind="ExternalInput")
    wo_d = nc.dram_tensor("wo2", [P, DIM], bf16, kind="ExternalInput")
    relx_d = nc.dram_tensor("relx2", [P, WER], bf16, kind="ExternalInput")
    edge_d = nc.dram_tensor("edge2", [P, 2], f32, kind="ExternalInput")
    ident_d = nc.dram_tensor("ident", [P, P], f32, kind="ExternalInput")
    y_d = nc.dram_tensor("y", [N, DIM], f32, kind="ExternalOutput")

    wer_d = [
        nc.dram_tensor(f"wer{h}", [N, WER], bf16, kind="Internal") for h in range(2)
    ]

    SCALE = float(D) ** -0.5
    NT = N // P  # 16 key tiles
    QW = 1024  # q-chunk width for the flash loop

    with tile.TileContext(nc) as tc:
        import contextlib

        ctx = contextlib.ExitStack()
        with ctx:
            const = ctx.enter_context(tc.tile_pool(name="const", bufs=1))
            big = ctx.enter_context(tc.tile_pool(name="big", bufs=1))
            cpool = ctx.enter_context(tc.tile_pool(name="copies", bufs=4))
            # PSUM budget (8 banks): zt pool 2x[128,1024]=4 + gpool 4x 1-bank=4.
            # zt has its own pool so flash never blocks on the pre-flash phases.
            ppool = ctx.enter_context(tc.tile_pool(name="ps", bufs=2, space="PSUM"))
            opool = ctx.enter_context(tc.tile_pool(name="ot", bufs=4, space="PSUM"))
            apool = ctx.enter_context(tc.tile_pool(name="attn", bufs=3))
            wpool = ctx.enter_context(tc.tile_pool(name="win", bufs=12))
            spool = ctx.enter_context(tc.tile_pool(name="small", bufs=2))
            dpool = ctx.enter_context(tc.tile_pool(name="dens", bufs=2))
            rbpool = ctx.enter_context(tc.tile_pool(name="rbp", bufs=2))

            # ---- load constants / inputs (small weights first) ----
            wq_sb = const.tile([P, 4, P], bf16)
            nc.sync.dma_start(wq_sb[:], wq_d.ap().rearrange("(c p) m -> p c m", p=P))
            wk_sb = const.tile([P, 4, P], bf16)
            nc.sync.dma_start(wk_sb[:], wk_d.ap().rearrange("(c p) m -> p c m", p=P))
            wv_sb = const.tile([P, 4, P], bf16)
            nc.sync.dma_start(wv_sb[:], wv_d.ap().rearrange("(c p) m -> p c m", p=P))
            wo_sb = const.tile([64, 2, DIM], bf16)
            nc.sync.dma_start(wo_sb[:], wo_d.ap().rearrange("(h p) m -> p h m", p=64))
            relx_sb = const.tile([P, WER], bf16)
            nc.sync.dma_start(relx_sb[:], relx_d.ap())
            edge_sb = const.tile([P, 2], f32)
            nc.sync.dma_start(edge_sb[:], edge_d.ap())
            ident_sb = const.tile([P, P], f32)
            nc.sync.dma_start(ident_sb[:], ident_d.ap())
            zero_sb = const.tile([P, P], bf16)
            nc.vector.memset(zero_sb[:], 0.0)
            den_pack = const.tile([97, 2, 512], f32)
            nc.vector.memset(den_pack[:], 1.0)
            ones64 = const.tile([P, 64], f32)
            nc.vector.memset(ones64[:], 1.0)

            # xT in 4 column chunks so projections can start early
            xt_sb = big.tile([P, 4, N], bf16)
            for ch in range(4):
                nc.sync.dma_start(
                    xt_sb[:, :, ch * 512 : (ch + 1) * 512],
                    xT_d.ap()[:, ch * 512 : (ch + 1) * 512].rearrange(
                        "(c p) n -> p c n", p=P
                    ),
                )

            # wER columns [0,128) are identically zero: write them once (on
            # the idle gpsimd queue, so the xT load is not delayed).
            for h in range(2):
                for qt in range(NT):
                    nc.gpsimd.dma_start(
                        wer_d[h].ap()[qt * P : (qt + 1) * P, 0:128], zero_sb[:]
                    )

            # ---- projections: qT2/kT2 (2 heads packed on partitions) ----
            qt2 = big.tile([P, N], bf16)
            ktp = big.tile([P, N], bf16)
            ktf = big.tile([P, N], bf16)
            kt_raw = big.tile([P, N], bf16)
            # Interleave q/k/v projections per n-chunk so PE work tracks the
            # chunked xT arrival instead of stalling on later chunks.
            v2 = big.tile([P, 2, NT, 65], bf16)
            nc.vector.memset(v2[:], 1.0)
            for nch in range(4):
                for dst, wsb in ((qt2, wq_sb), (kt_raw, wk_sb)):
                    pt = opool.tile([P, 512], f32, name="proj", tag="outT")
                    for cc in range(4):
                        nc.tensor.matmul(
                            pt[:],
                            wsb[:, cc, :],
                            xt_sb[:, cc, nch * 512 : (nch + 1) * 512],
                            start=(cc == 0),
                            stop=(cc == 3),
                        )
                    nc.vector.tensor_copy(
                        dst[:, nch * 512 : (nch + 1) * 512], pt[:]
                    )
                for kt in range(nch * 4, nch * 4 + 4):
                    pt = opool.tile([P, 512], f32, name="vproj", tag="outT")
                    for cc in range(4):
                        nc.tensor.matmul(
                            pt[:, :P],
                            xt_sb[:, cc, kt * P : (kt + 1) * P],
                            wv_sb[:, cc, :],
                            start=(cc == 0),
                            stop=(cc == 3),
                        )
                    for h in range(2):
                        nc.vector.tensor_copy(
                            v2[:, h, kt, 0:64], pt[:, h * 64 : h * 64 + 64]
                        )

            nc.vector.tensor_scalar_add(ktp[:], kt_raw[:], edge_sb[:, 0:1])
            nc.vector.tensor_scalar_add(ktf[:], kt_raw[:], edge_sb[:, 1:2])

            # ---- wER tables -> DRAM.
            # relX' columns [0,128) are exactly zero (pre-zeroed above);
            # [1152,1280) give the future-minus-past edge residual.
            # h0's tables are emitted before flash(0,0); h1's groups are
            # interleaved into flash(0,0)'s kt loop as background work so the
            # PE stream stays dense and flash starts ~15us earlier.
            def wer_group(h, qt):
                hs = slice(h * 64, h * 64 + 64)
                wtile = cpool.tile([P, 1152], bf16, name="wer_sb", tag="wer_sb")
                ptA1 = opool.tile([P, 512], f32, name="wer_psA1", tag="outT")
                nc.tensor.matmul(
                    ptA1[:],
                    qt2[hs, qt * P : (qt + 1) * P],
                    relx_sb[hs, 128:640],
                    start=True,
                    stop=True,
                )
                ptA2 = opool.tile([P, 512], f32, name="wer_psA2", tag="outT")
                nc.tensor.matmul(
                    ptA2[:],
                    qt2[hs, qt * P : (qt + 1) * P],
                    relx_sb[hs, 640:1152],
                    start=True,
                    stop=True,
                )
                nc.vector.tensor_copy(wtile[:, 0:512], ptA1[:])
                if h == 0:
                    nc.scalar.copy(wtile[:, 512:1024], ptA2[:])
                else:
                    nc.vector.tensor_copy(wtile[:, 512:1024], ptA2[:])
                ptB = opool.tile([P, 512], f32, name="wer_psB", tag="outT")
                nc.tensor.matmul(
                    ptB[:, 0:128],
                    qt2[hs, qt * P : (qt + 1) * P],
                    relx_sb[hs, 1152:1280],
                    start=True,
                    stop=True,
                )
                nc.vector.tensor_copy(wtile[:, 1024:1152], ptB[:, 0:128])
                nc.sync.dma_start(
                    wer_d[h].ap()[qt * P : (qt + 1) * P, 128:1280], wtile[:]
                )

            for qt in range(NT):
                wer_group(0, qt)

            # ---- flash attention (qc outer so the tail pipelines) ----
            otn = big.tile([64, 2, N], bf16)  # normalized outT per head
            ostages = {}

            def flash(qc, h, bg=()):
                bg = list(bg)
                hs = slice(h * 64, h * 64 + 64)
                if h == 0:
                    ostages[qc] = spool.tile(
                        [65, 4, 512], f32, name="ostage", tag="ostage"
                    )
                ostage = ostages[qc]
                # Window bias tiles: diagonally-gathered rows of the wER
                # table (contiguous 2.3KB per-row reads, cast bf16->f32 by
                # the SWDGE path); consumed by PE transpose-matmuls.
                wins = []
                for j in range(QW // P):
                    qb = qc * QW + j * P
                    r0 = max(0, qb - 512)
                    r1 = min(N, qb + 640)
                    rw = r1 - r0
                    wt = wpool.tile([P, 1152], f32, name=f"win{j}", tag="win")
                    wsrc = AP(
                        tensor=wer_d[h],
                        offset=qb * (WER - 1) + 640 + r0,
                        ap=[[WER - 1, P], [1, rw]],
                    )
                    nc.gpsimd.dma_start(wt[:, :rw], wsrc)
                    wins.append((wt, r0))

                oth = [
                    opool.tile([65, 512], f32, name=f"outT{half}", tag="outT")
                    for half in range(2)
                ]

                # AV matmuls are emitted one kt behind the score matmuls:
                # the PE queue is strict FIFO, so an AV right after its own
                # kt's scores would stall the PE on exp(kt) (ACT) every
                # iteration. One-deep software pipelining keeps PE streaming.
                def emit_av(kt, at):
                    for half in range(2):
                        nc.tensor.matmul(
                            oth[half][:],
                            v2[:, h, kt, :],
                            at[:, half * 512 : (half + 1) * 512],
                            start=(kt == 0),
                            stop=(kt == NT - 1),
                        )

                pending_av = None
                for kt in range(NT):
                    kb = kt * P
                    zt = ppool.tile([P, QW], f32, name="zt", tag="ps")
                    # class per 128-block: fully-future -> ktf, else ktp
                    cls = []
                    for j in range(QW // P):
                        dlt = qc * QW + j * P - kb
                        cls.append("f" if dlt <= -640 else "u")
                    for half in range(QW // 512):
                        j0 = half * 4
                        runs = []
                        for j in range(j0, j0 + 4):
                            if runs and runs[-1][2] == cls[j]:
                                runs[-1][1] += P
                            else:
                                runs.append([j * P, P, cls[j]])
                        first = True
                        for s, wd, c in runs:
                            kvar = ktf if c == "f" else ktp
                            nc.tensor.matmul(
                                zt[:, s : s + wd],
                                kvar[hs, kb : kb + P],
                                qt2[hs, qc * QW + s : qc * QW + s + wd],
                                start=first,
                                stop=False,
                                skip_group_check=True,
                            )
                            first = False
                        for j in range(j0, j0 + 4):
                            dlt = qc * QW + j * P - kb
                            if abs(dlt) >= 640:
                                continue
                            wt, r0 = wins[j]
                            nc.tensor.matmul(
                                zt[:, j * P : (j + 1) * P],
                                wt[:, kb - r0 : kb - r0 + P],
                                ident_sb[:],
                                is_transpose=True,
                                start=False,
                                stop=False,
                                skip_group_check=True,
                            )
                    at = apool.tile([P, QW], bf16, name="attnT")
                    nc.scalar.activation(
                        at[:], zt[:], mybir.ActivationFunctionType.Exp,
                        scale=SCALE,
                    )
                    if pending_av is not None:
                        emit_av(*pending_av)
                    pending_av = (kt, at)
                    # one unit of background work (h1 wER groups) per kt
                    if bg:
                        bg.pop(0)()
                emit_av(*pending_av)
                for half in range(2):
                    slot = h * 2 + half
                    nc.vector.tensor_copy(ostage[:, slot, :], oth[half][:])

            def tail(qc):
                # Pack the 4 denominator rows at partitions 0/32/64/96 so one
                # batched reciprocal covers them (recip cost is per-partition-
                # element; a [1,512] alone costs the same 3.4us). The
                # broadcast back to 64 partitions rides the idle GpSimd.
                ostage = ostages.pop(qc)
                for slot in range(4):
                    nc.sync.dma_start(
                        den_pack[32 * slot : 32 * slot + 1, qc, :],
                        ostage[64:65, slot, :],
                    )
                rden_pack = dpool.tile([97, 512], f32, name="rdenp", tag="den")
                nc.vector.reciprocal(rden_pack[:], den_pack[:, qc, :])
                for slot in range(4):
                    h, half = slot // 2, slot % 2
                    rb = opool.tile([64, 512], f32, name="recipb", tag="outT")
                    nc.tensor.matmul(
                        rb[:],
                        ones64[32 * slot : 32 * slot + 1, :],
                        rden_pack[32 * slot : 32 * slot + 1, :],
                        start=True,
                        stop=True,
                        tile_position=(32 * slot, 0),
                    )
                    q0 = qc * QW + half * 512
                    nc.vector.tensor_mul(
                        otn[:, h, q0 : q0 + 512], ostage[0:64, slot, :], rb[:]
                    )
                for nt8 in range(QW // P):
                    nt = qc * (QW // P) + nt8
                    pt = opool.tile([P, 512], f32, name="yproj", tag="outT")
                    for h in range(2):
                        nc.tensor.matmul(
                            pt[:],
                            otn[:, h, nt * P : (nt + 1) * P],
                            wo_sb[:, h, :],
                            start=(h == 0),
                            stop=(h == 1),
                        )
                    yt = cpool.tile([P, 512], f32, name="y_sb")
                    nc.vector.tensor_copy(yt[:], pt[:])
                    nc.sync.dma_start(y_d.ap()[nt * P : (nt + 1) * P, :], yt[:])

            # tail(0) is emitted after flash(1,0) so the PE never waits on the
            # reciprocal chain (PE executes strictly in emission order).
            flash(0, 0, bg=[
                (lambda qt=qt: wer_group(1, qt)) for qt in range(NT)
            ])
            flash(0, 1)
            flash(1, 0)
            tail(0)
            flash(1, 1)
            tail(1)

    nc.compile()
    return nc


def _host_prep(x, Wq, Wkv, Wo, rel_emb):
    """Build the 8 per-core input maps."""
    import ml_dtypes

    bf = ml_dtypes.bfloat16
    ident = np.eye(P, dtype=np.float32)
    relX = (
        rel_emb[np.clip(1152 - np.arange(WER), 0, 1024)] - rel_emb[1024][None, :]
    ).T
    relx2 = np.ascontiguousarray(np.concatenate([relX, relX], axis=0).astype(bf))
    edge = np.stack([rel_emb[1024], rel_emb[0]], axis=1)
    edge2 = np.ascontiguousarray(
        np.concatenate([edge, edge], axis=0).astype(np.float32)
    )
    Wkv_r = Wkv.reshape(DIM, 2, HEADS, D)
    in_maps = []
    for core in range(8):
        b = core // 4
        h0 = 2 * (core % 4)
        in_maps.append(
            {
                "xT": np.ascontiguousarray(x[b].T.astype(bf)),
                "wq2": np.ascontiguousarray(Wq[:, h0 * D : (h0 + 2) * D].astype(bf)),
                "wk2": np.ascontiguousarray(
                    Wkv_r[:, 0, h0 : h0 + 2].reshape(DIM, 2 * D).astype(bf)
                ),
                "wv2": np.ascontiguousarray(
                    Wkv_r[:, 1, h0 : h0 + 2].reshape(DIM, 2 * D).astype(bf)
                ),
                "wo2": np.ascontiguousarray(
                    Wo[h0 * D : (h0 + 2) * D, :].astype(bf)
                ),
                "relx2": relx2,
                "edge2": edge2,
                "ident": ident,
            }
        )
    return in_maps


def kernel(x, Wq, Wkv, Wo, bo, rel_emb, _want_trace=False):
    from concourse.bass_utils import run_bass_kernel_spmd

    x = np.asarray(x)
    if "nc" not in _cached:
        _cached["nc"] = _build_program()
    nc = _cached["nc"]
    in_maps = _host_prep(x, np.asarray(Wq), np.asarray(Wkv), np.asarray(Wo),
                         np.asarray(rel_emb))
    res = run_bass_kernel_spmd(
        nc, in_maps, core_ids=list(range(8)), trace=_want_trace
    )
    _cached["last_result"] = res
    y = np.zeros((2, N, DIM), np.float32)
    for core in range(8):
        y[core // 4] += res.results[core]["y"]
    y += np.asarray(bo).astype(np.float32)[None, None, :]
    return y
